# revision 62
# baseline (speedup 1.0000x reference)
"""MoNet (GMMConv GNN) distributed Trainium2 kernel — source-partitioned.

Strategy (8 NeuronCores):
  - Edges partitioned by SOURCE core (row // 6250): each core computes xg for
    its local nodes only (no xg AllGather) and gathers source rows from its
    OWN small table (6272 rows, int16 indices, one SWDGE index space).
  - Per layer: local xg = h @ Wg written to a local DRAM table (bf16,
    128-col rows = 256B); edges sorted by global dest block (448 blocks of
    112 dests); per 128-edge tile one gaussian-weighted one-hot matmul
    scatters into a [96 feat x 112 dest] PSUM accumulator (transposed
    layout, so no PE transposes anywhere); per dest-group staging is DMAd to
    a partial-aggregate buffer [8*96, 6272] and a single bf16 ReduceScatter
    (add) replaces the baseline's 25MB AllGather.
  - Root weight + conv bias are folded into one K=97 matmul (h carries a
    ones row); the reduce-scattered aggregate is injected into the same PSUM
    via an identity matmul; epilogue is relu + residual add in-place.
  - All matmuls/tables bf16 (PSUM accumulation f32); gaussian edge
    coefficients computed in f32.
  - Host does index prep only: degree/dinv, edge bucketing/padding.
"""

import sys
from contextlib import ExitStack

import numpy as np

if "/opt/trn_rl_repo" not in sys.path:
    sys.path.insert(0, "/opt/trn_rl_repo")

import ml_dtypes

import concourse.bacc as bacc
import concourse.mybir as mybir
import concourse.tile as tile
from concourse import bass_utils, library_config

F32 = mybir.dt.float32
BF16 = mybir.dt.bfloat16
I16 = mybir.dt.int16
AF = mybir.ActivationFunctionType
ALU = mybir.AluOpType

P = 128
EPS = 1e-15
BF = ml_dtypes.bfloat16


class Cfg:
    def __init__(self):
        self.N, self.E = 50000, 800000
        self.NFEAT, self.NHID, self.NCLASS, self.NL, self.C = 128, 96, 40, 2, 8
        self.B = self.N // self.C            # 6250 real nodes per core
        self.BS = 224                        # dest block size
        self.BPG = 28                        # blocks per group (core)
        self.NPH = 4                         # RS pipeline phases
        self.HB = self.BPG // self.NPH       # 7 blocks per phase
        self.Bp = self.BS * self.BPG         # 6272 padded nodes per core
        self.Bh = self.BS * self.HB          # 1568 cols per phase
        self.NGB = self.C * self.BPG         # 448 global dest blocks
        self.NSB = self.Bp // P              # 49 source 128-blocks
        self.TPC = 7                         # gather tiles per SWDGE call
        self.RING = 49152                    # 3072-descriptor SWDGE ring
        self.SKIP_RS = False                 # debug: replace ReduceScatter
        self.SKIP_GATHER = False             # debug: skip dma_gather calls


def host_prep(cfg, edge_index, edge_weight):
    """Bucket edges by (dest half, source core, global dest block); pad tiles
    to the max count over cores so the SPMD program structure is uniform.
    Blocks are ordered (half, group, k) so each half's tiles are contiguous
    and the first half's ReduceScatter can overlap the second half's math."""
    N, C, B, BS, BPG, NGB = cfg.N, cfg.C, cfg.B, cfg.BS, cfg.BPG, cfg.NGB
    HB = cfg.HB
    row = np.asarray(edge_index[0]).astype(np.int64)
    col = np.asarray(edge_index[1]).astype(np.int64)
    ew = np.asarray(edge_weight).astype(np.float64)
    deg = np.bincount(row, weights=ew, minlength=N).astype(np.float64)
    with np.errstate(divide="ignore"):
        dinv = np.where(deg > 0, 1.0 / np.sqrt(deg), 0.0).astype(np.float32)

    core = row // B
    src_loc = row - core * B
    g = col // B
    dlg = col - g * B
    kblk = dlg // BS
    lane_d = (dlg - kblk * BS).astype(np.float32)
    ph = kblk // HB
    # phase-ordered slot: (phase, group, k within phase)
    gb = ph * (C * HB) + g * HB + (kblk - ph * HB)

    order = np.lexsort((gb, core))
    core, gb = core[order], gb[order]
    src_loc, lane_d = src_loc[order], lane_d[order]
    u = dinv[row[order]]
    v = dinv[col[order]]

    cnt = np.zeros((C, NGB), np.int64)
    np.add.at(cnt, (core, gb), 1)
    K = ((cnt + P - 1) // P).max(axis=0)          # tiles per slot
    toff = np.concatenate([[0], np.cumsum(K)]).astype(np.int64)
    T = int(toff[-1])

    gg = core * NGB + gb
    gcnt = np.bincount(gg, minlength=C * NGB)
    gstart = np.concatenate([[0], np.cumsum(gcnt)])[:-1]
    idx_in_g = np.arange(len(gg)) - gstart[gg]
    lane = (idx_in_g % P).astype(np.int64)
    t = (toff[gb] + idx_in_g // P).astype(np.int64)

    edA = np.zeros((C, P, 3 * T), np.float32)
    edA[:, :, 2 * T:3 * T] = -1.0                 # dl sentinel: no dest match
    edA[core, lane, t] = u
    edA[core, lane, T + t] = v
    edA[core, lane, 2 * T + t] = lane_d

    # int16 idx, wrapped-16: element (t, lane) at [lane % 16, t*8 + lane//16]
    idxA = np.zeros((C, 16, 8 * T), np.int16)     # pad idx 0 (valid row)
    idxA[core, lane % 16, t * 8 + lane // 16] = src_loc.astype(np.int16)
    idxA = np.tile(idxA, (1, 8, 1))
    return dict(idxA=idxA, edA=edA, K=[int(x) for x in K],
                toff=[int(x) for x in toff], T=T)


def make_scal(cfg, Wp, bp, mu, sigma):
    Wp = np.asarray(Wp, np.float64)
    bp = np.asarray(bp, np.float64)
    mu = np.asarray(mu, np.float64)
    sigma = np.asarray(sigma, np.float64)
    out = []
    for i in range(cfg.NL):
        out.append(dict(
            wp0=float(Wp[i, 0, 0]),
            wp1=float(Wp[i, 1, 0]),
            bp=float(bp[i, 0]),
            neg_mu=float(-mu[i, 0, 0]),
            s2inv=float(-0.5 / (EPS + sigma[i, 0, 0] ** 2)),
        ))
    return out


def build(cfg, prep, scal):
    NHID, NCLASS, NL, C = cfg.NHID, cfg.NCLASS, cfg.NL, cfg.C
    BS, BPG, Bp, NGB, NSB, TPC = cfg.BS, cfg.BPG, cfg.Bp, cfg.NGB, cfg.NSB, cfg.TPC
    HB, Bh = cfg.HB, cfg.Bh
    K2, toff, T = prep["K"], prep["toff"], prep["T"]

    nc = bacc.Bacc("TRN2", target_bir_lowering=False, debug=False,
                   num_devices=C, dynamic_dma_scratch_size=cfg.RING)
    hT_in = nc.declare_dram_parameter("hT", [P, Bp], BF16, isOutput=False)
    idx_in = nc.declare_dram_parameter("idx16", [P, 8 * T], I16, isOutput=False)
    ed_in = nc.declare_dram_parameter("ed", [P, 3 * T], F32, isOutput=False)
    ri_in = nc.declare_dram_parameter("riota", [P, BS], BF16, isOutput=False)
    Wemb_in = nc.declare_dram_parameter("Wemb", [P, NHID], BF16, isOutput=False)
    Wg_in = nc.declare_dram_parameter("WgP", [NL, NHID, P], BF16, isOutput=False)
    Wr_in = nc.declare_dram_parameter("Wr", [NL, NHID, NHID], BF16, isOutput=False)
    Wo_in = nc.declare_dram_parameter("Wo", [NHID, NCLASS], BF16, isOutput=False)
    bemb_in = nc.declare_dram_parameter("bembT", [NHID, 1], F32, isOutput=False)
    bconv_in = nc.declare_dram_parameter("bconvT", [NHID, NL], F32, isOutput=False)
    out_ext = nc.declare_dram_parameter("out", [Bp, NCLASS], F32, isOutput=True)

    with tile.TileContext(nc) as tc, ExitStack() as ctx:
        nc.gpsimd.load_library(library_config.mlp)
        const = ctx.enter_context(tc.tile_pool(name="const", bufs=1))
        hp = ctx.enter_context(tc.tile_pool(name="hp", bufs=2))
        gtp = ctx.enter_context(tc.tile_pool(name="gtp", bufs=2))
        gaussp = ctx.enter_context(tc.tile_pool(name="gaussp", bufs=1))
        xsp = ctx.enter_context(tc.tile_pool(name="xsp", bufs=1))
        xjp = ctx.enter_context(tc.tile_pool(name="xjp", bufs=4))
        selp = ctx.enter_context(tc.tile_pool(name="selp", bufs=16))
        stp = ctx.enter_context(tc.tile_pool(name="stp", bufs=3))
        agp = ctx.enter_context(tc.tile_pool(name="agp", bufs=4))
        rootp = ctx.enter_context(tc.tile_pool(name="rootp", bufs=2))
        rlp = ctx.enter_context(tc.tile_pool(name="rlp", bufs=4))
        obp = ctx.enter_context(tc.tile_pool(name="obp", bufs=1))
        pmm = ctx.enter_context(tc.tile_pool(name="pmm", bufs=3, space="PSUM"))
        pagg = ctx.enter_context(tc.tile_pool(name="pagg", bufs=4, space="PSUM"))
        prt = ctx.enter_context(tc.tile_pool(name="prt", bufs=1, space="PSUM"))
        dramp = ctx.enter_context(tc.tile_pool(name="dramp", bufs=1, space="DRAM"))

        def cload(ap, shape, dtype=F32, name=None):
            tl = const.tile(shape, dtype, name=name or "c")
            nc.sync.dma_start(out=tl[:], in_=ap)
            return tl

        hTin_s = cload(hT_in[:, :], [P, Bp], BF16, "hTin_s")
        idx_s = cload(idx_in[:, :], [P, 8 * T], I16, "idx_s")
        ed_s = cload(ed_in[:, :], [P, 3 * T], F32, "ed_s")
        u_s = ed_s[:, 0:T]
        v_s = ed_s[:, T:2 * T]
        dl_s = ed_s[:, 2 * T:3 * T]
        ri_s = cload(ri_in[:, :], [P, BS], BF16, "ri_s")
        Wemb_s = cload(Wemb_in[:, :], [P, NHID], BF16, "Wemb_s")
        bemb_s = cload(bemb_in[:, :], [NHID, 1], F32, "bemb_s")
        Wo_s = cload(Wo_in[:, :], [NHID, NCLASS], BF16, "Wo_s")
        bconv_s = cload(bconv_in[:, :], [NHID, NL], F32, "bconv_s")
        Wg_s = const.tile([NHID, NL * P], BF16, name="Wg_s")
        Wr_s = const.tile([NHID, NL * NHID], BF16, name="Wr_s")
        for i in range(NL):
            nc.sync.dma_start(out=Wg_s[:, i * P:(i + 1) * P], in_=Wg_in[i])
            nc.sync.dma_start(out=Wr_s[:, i * NHID:(i + 1) * NHID], in_=Wr_in[i])

        # ---- embedding: h0T[96, Bp] = (h @ Wemb + bemb).T ----
        # 4 node-blocks share one PSUM bank so each Act copy moves 512 cols.
        h_cur = hp.tile([NHID, Bp], BF16, tag="h", name="h0")
        for q in range(0, NSB, 4):
            nb = min(4, NSB - q)
            pe = pmm.tile([P, 4 * P], F32, tag="mm2", name="pe")
            for b in range(nb):
                c0 = (q + b) * P
                nc.tensor.matmul(pe[:NHID, b * P:(b + 1) * P], lhsT=Wemb_s[:],
                                 rhs=hTin_s[:, c0:c0 + P], start=True, stop=True)
            if (q // 4) % 2 == 0:
                nc.scalar.activation(out=h_cur[:, q * P:q * P + nb * P],
                                     in_=pe[:NHID, :nb * P],
                                     func=AF.Identity, bias=bemb_s[:, 0:1])
            else:
                nc.vector.tensor_scalar(out=h_cur[:, q * P:q * P + nb * P],
                                        in0=pe[:NHID, :nb * P],
                                        scalar1=bemb_s[:, 0:1], scalar2=None,
                                        op0=ALU.add)

        # ---- layers ----
        gauss_l = []
        pending_epi3 = None
        QSPL = 36  # xg/head blocks below this need only epi phases 0-2
        for li in range(NL):
            # local xg table -> DRAM [Bp, 128] bf16 (4 blocks per PSUM bank).
            # Emitted in two parts around the previous layer's phase-3
            # epilogue so part A runs while that layer's last RS is in flight.
            xgstage = xsp.tile([P, NSB * P], BF16, tag="xgs", name="xgs")

            def xg_part(qr, li=li, xgstage=xgstage, h_cur=h_cur):
                for q in qr:
                    nb = min(4, NSB - q)
                    px = pmm.tile([P, 4 * P], F32, tag="mm2", name="px")
                    for b in range(nb):
                        c0 = (q + b) * P
                        nc.tensor.matmul(px[:, b * P:(b + 1) * P],
                                         lhsT=h_cur[:, c0:c0 + P],
                                         rhs=Wg_s[:, li * P:(li + 1) * P],
                                         start=True, stop=True)
                    if (q // 4) % 2 == 0:
                        nc.scalar.copy(out=xgstage[:, q * P:q * P + nb * P],
                                       in_=px[:, :nb * P])
                    else:
                        nc.vector.tensor_copy(
                            out=xgstage[:, q * P:q * P + nb * P],
                            in_=px[:, :nb * P])

            xg_part(range(0, QSPL, 4))
            xg_d = dramp.tile([Bp, P], BF16, tag=f"xg{li}", name=f"xg{li}")
            nc.sync.dma_start(
                out=xg_d[0:QSPL * P, :].rearrange("(a p) c -> p a c", p=P),
                in_=xgstage[:, 0:QSPL * P].rearrange("p (a c) -> p a c", c=P))
            if pending_epi3 is not None:
                pending_epi3()
            xg_part(range(QSPL, NSB, 4))
            nc.sync.dma_start(
                out=xg_d[QSPL * P:, :].rearrange("(a p) c -> p a c", p=P),
                in_=xgstage[:, QSPL * P:].rearrange("p (a c) -> p a c", c=P))

            def emit_gauss(lj):
                # gaussian edge coefficients (only need ed); the elementwise
                # chain runs on the otherwise-idle gpsimd.
                if True:
                    sc = scal[lj]
                    t1 = gtp.tile([P, T], F32, tag="g1", name="g1")
                    nc.gpsimd.tensor_scalar(out=t1[:], in0=v_s[:],
                                            scalar1=sc["wp1"], scalar2=sc["bp"],
                                            op0=ALU.mult, op1=ALU.add)
                    t2 = gtp.tile([P, T], F32, tag="g2", name="g2")
                    nc.gpsimd.tensor_scalar(out=t2[:], in0=u_s[:],
                                            scalar1=sc["wp0"],
                                            scalar2=None, op0=ALU.mult)
                    t3 = gtp.tile([P, T], F32, tag="g1", name="g3")
                    nc.gpsimd.tensor_tensor(out=t3[:], in0=t1[:], in1=t2[:],
                                            op=ALU.add)
                    t4 = gtp.tile([P, T], F32, tag="g2", name="g4")
                    nc.scalar.activation(out=t4[:], in_=t3[:], func=AF.Tanh)
                    t4b = gtp.tile([P, T], F32, tag="g1", name="g4b")
                    nc.gpsimd.tensor_scalar(out=t4b[:], in0=t4[:],
                                            scalar1=sc["neg_mu"],
                                            scalar2=None, op0=ALU.add)
                    t5 = gtp.tile([P, T], F32, tag="g2", name="g5")
                    nc.scalar.activation(out=t5[:], in_=t4b[:], func=AF.Square)
                    g_s = gaussp.tile([P, T], F32, tag=f"gauss{lj}",
                                      name=f"gauss{lj}")
                    nc.scalar.activation(out=g_s[:], in_=t5[:], func=AF.Exp,
                                         scale=sc["s2inv"])
                    gauss_l.append(g_s)

            if li == 0:
                emit_gauss(0)
            gauss_s = gauss_l[li]

            # Per phase: gather calls emitted just before that phase's scatter
            # so the Pool stream reaches the RS instruction promptly and each
            # phase's RS overlaps the next phase's math. Epilogues are emitted
            # after ALL phases so they don't block the stream order.
            h_new = hp.tile([NHID, Bp], BF16, tag="h", name=f"h{li + 1}")
            tile_call = {}
            agg_half = []
            pending_rs = []

            def emit_rs():
                # deferred two phases so the Pool stream (which also carries
                # gather descriptor-gen) reaches the collective only after
                # its sem-wait on the flush DMAs is long satisfied — a
                # stalled Pool stops descriptor-gen and drains the DMA
                # pipeline. The SBUF load of the result is deferred to the
                # epilogue: a sync-queue DMA waiting on the collective would
                # hold SP.SEQ and block the later staging flushes.
                partial, hf_ = pending_rs.pop(0)
                if cfg.SKIP_RS:
                    agg_half.append(partial[0:NHID, :])
                    return
                aggrs_d = dramp.tile([NHID, Bh], BF16, tag=f"ag{li}h{hf_}",
                                     name=f"aggrs{li}h{hf_}")
                nc.gpsimd.collective_compute(
                    "ReduceScatter", ALU.add,
                    replica_groups=[list(range(C))],
                    ins=[partial[:, :]],
                    outs=[aggrs_d[:, :]],
                )
                agg_half.append(aggrs_d)

            for hf in range(cfg.NPH):
                ta = toff[hf * C * HB]
                tb = toff[(hf + 1) * C * HB]
                t0 = ta
                while t0 < tb:
                    kc = min(TPC, tb - t0)
                    xj = xjp.tile([P, TPC * P], BF16, tag="xj", name="xj")
                    if cfg.SKIP_GATHER:
                        nc.vector.memset(xj[:], 0.0)
                    else:
                        out_ap = xj[:, :kc * P].rearrange("p (k e) -> p k e", e=P)
                        nc.gpsimd.dma_gather(out_ap, xg_d[:, :],
                                             idx_s[:, t0 * 8:(t0 + kc) * 8],
                                             kc * P, kc * P, P)
                    for s in range(kc):
                        tile_call[t0 + s] = (xj, s)
                    t0 += kc
                while pending_rs:
                    emit_rs()

                partial_d = dramp.tile([C * NHID, Bh], BF16, tag=f"pt{li}h{hf}",
                                       name=f"partial{li}h{hf}")
                ncopy = 0
                for grp in range(C):
                    stg = stp.tile([NHID, Bh], BF16, tag="stg", name="stg")
                    for k in range(HB):
                        slot = hf * (C * HB) + grp * HB + k
                        Kb = K2[slot]
                        if Kb == 0:
                            nc.vector.memset(stg[:, k * BS:(k + 1) * BS], 0.0)
                            continue
                        pa = pagg.tile([NHID, BS], F32, tag="pa", name="pa")
                        for j in range(Kb):
                            t = toff[slot] + j
                            sel = selp.tile([P, BS], BF16, tag="sel", name="sel")
                            nc.vector.tensor_scalar(
                                out=sel[:], in0=ri_s[:],
                                scalar1=dl_s[:, t:t + 1],
                                scalar2=gauss_s[:, t:t + 1],
                                op0=ALU.is_equal, op1=ALU.mult)
                            xj, sl = tile_call[t]
                            nc.tensor.matmul(
                                pa[:, :],
                                lhsT=xj[:, sl * P:sl * P + NHID],
                                rhs=sel[:], start=(j == 0), stop=(j == Kb - 1))
                        # alternate copy engine 2:1 to balance Act/DVE load
                        if ncopy % 3 != 2:
                            nc.scalar.copy(out=stg[:, k * BS:(k + 1) * BS],
                                           in_=pa[:, :])
                        else:
                            nc.vector.tensor_copy(out=stg[:, k * BS:(k + 1) * BS],
                                                  in_=pa[:, :])
                        ncopy += 1
                    fl = nc.sync.dma_start(
                        out=partial_d[grp * NHID:(grp + 1) * NHID, :],
                        in_=stg[:, :])
                pending_rs.append((partial_d, hf))
                if li == 0 and hf == 0:
                    emit_gauss(1)   # layer-1 coefficients during the scatter
            last_flush = fl
            while pending_rs:
                emit_rs()

            # root term: rootT = (h @ Wroot + bconv).T — emitted after the
            # scatter so its Act copies don't clog the front of the Act
            # queue (they run while the reduce-scatters are in flight).
            root_sb = rootp.tile([NHID, Bp], BF16, tag="root", name=f"root{li}")
            for q in range(0, BPG, 2):
                pr = prt.tile([NHID, 2 * BS], F32, tag="rt", name="pr")
                for b in range(2):
                    c0 = (q + b) * BS
                    nc.tensor.matmul(pr[:, b * BS:(b + 1) * BS],
                                     lhsT=Wr_s[:, li * NHID:(li + 1) * NHID],
                                     rhs=h_cur[:, c0:c0 + BS],
                                     start=True, stop=True)
                nc.scalar.activation(out=root_sb[:, q * BS:(q + 2) * BS],
                                     in_=pr[:, :], func=AF.Identity,
                                     bias=bconv_s[:, li:li + 1])

            # epilogues (early phases overlap the later RSs); phase 3 is
            # deferred into the NEXT layer's xg section:
            # h_new = h_cur + relu(root + agg)
            def emit_epi(hf, agg_half=agg_half, root_sb=root_sb,
                         h_new=h_new, h_cur=h_cur, last_flush=last_flush):
                aggsb = agp.tile([NHID, Bh], BF16, tag="agg", name="aggsb")
                ld = nc.sync.dma_start(out=aggsb[:, :], in_=agg_half[hf][:, :])
                # order this load AFTER the last staging flush: it waits on
                # the collective while holding SP.SEQ, which would otherwise
                # block the remaining flush DMAs queued behind it.
                ld.ins.add_dependency(
                    last_flush.ins.name,
                    mybir.DependencyInfo(sync=True, no_sync=False))
                hc0 = hf * Bh
                for k in range(HB):
                    c0 = hc0 + k * BS
                    sm = rlp.tile([NHID, BS], BF16, tag="sm", name="sm")
                    nc.vector.tensor_tensor(out=sm[:, :],
                                            in0=aggsb[:, k * BS:(k + 1) * BS],
                                            in1=root_sb[:, c0:c0 + BS], op=ALU.add)
                    rl = rlp.tile([NHID, BS], BF16, tag="rl", name="rl")
                    if k % 2 == 0:
                        nc.scalar.activation(out=rl[:, :], in_=sm[:, :],
                                             func=AF.Relu)
                    else:
                        nc.vector.tensor_scalar(out=rl[:, :], in0=sm[:, :],
                                                scalar1=0.0, scalar2=None,
                                                op0=ALU.max)
                    nc.vector.tensor_tensor(out=h_new[:, c0:c0 + BS],
                                            in0=rl[:, :],
                                            in1=h_cur[:, c0:c0 + BS],
                                            op=ALU.add)

            for hf in range(cfg.NPH - 1):
                emit_epi(hf)
            pending_epi3 = lambda f=emit_epi: f(cfg.NPH - 1)
            h_cur = h_new

        # ---- output head (4 blocks per PSUM bank); split around the last
        # layer's deferred phase-3 epilogue ----
        ob = obp.tile([P, NSB * NCLASS], F32, tag="ob", name="ob")

        def head_part(qr):
            for q in qr:
                nb = min(4, NSB - q)
                po = pmm.tile([P, 4 * P], F32, tag="mm2", name="po")
                for b in range(nb):
                    c0 = (q + b) * P
                    nc.tensor.matmul(po[:, b * NCLASS:(b + 1) * NCLASS],
                                     lhsT=h_cur[:, c0:c0 + P], rhs=Wo_s[:],
                                     start=True, stop=True)
                if (q // 4) % 2 == 0:
                    nc.scalar.copy(out=ob[:, q * NCLASS:(q + nb) * NCLASS],
                                   in_=po[:, :nb * NCLASS])
                else:
                    nc.vector.tensor_copy(
                        out=ob[:, q * NCLASS:(q + nb) * NCLASS],
                        in_=po[:, :nb * NCLASS])

        head_part(range(0, QSPL, 4))
        pending_epi3()
        head_part(range(QSPL, NSB, 4))
        nc.sync.dma_start(
            out=out_ext[:, :].rearrange("(a p) c -> p a c", p=P),
            in_=ob[:, :].rearrange("p (a c) -> p a c", c=NCLASS))

    nc.finalize()
    return nc


def make_in_maps(cfg, prep, h, W_emb, b_emb, Wg, Wroot, b_conv, W_out, b_out):
    C, B, Bp, NL = cfg.C, cfg.B, cfg.Bp, cfg.NL
    NHID, NCLASS, BS, P_ = cfg.NHID, cfg.NCLASS, cfg.BS, P
    h = np.asarray(h, np.float32)
    WgP = np.zeros((NL, NHID, P_), np.float32)
    WgP[:, :, :NHID] = np.asarray(Wg, np.float32).reshape(NL, NHID, NHID)
    riota = np.tile(np.arange(BS, dtype=np.float32), (P_, 1))
    common = dict(
        riota=np.ascontiguousarray(riota.astype(BF)),
        Wemb=np.ascontiguousarray(np.asarray(W_emb, np.float32).astype(BF)),
        WgP=np.ascontiguousarray(WgP.astype(BF)),
        Wr=np.ascontiguousarray(np.asarray(Wroot, np.float32).astype(BF)),
        Wo=np.ascontiguousarray(np.asarray(W_out, np.float32).astype(BF)),
        bembT=np.ascontiguousarray(np.asarray(b_emb, np.float32)[:, None]),
        bconvT=np.ascontiguousarray(np.asarray(b_conv, np.float32).T),
    )
    in_maps = []
    for m in range(C):
        d = dict(common)
        hT = np.zeros((P_, Bp), np.float32)
        hT[:, :B] = h[m * B:(m + 1) * B, :].T
        d["hT"] = np.ascontiguousarray(hT.astype(BF))
        d["idx16"] = np.ascontiguousarray(prep["idxA"][m])
        d["ed"] = np.ascontiguousarray(prep["edA"][m])
        in_maps.append(d)
    return in_maps


def run(cfg, inputs, trace=False):
    prep = host_prep(cfg, inputs["edge_index"], inputs["edge_weight"])
    scal = make_scal(cfg, inputs["Wp"], inputs["bp"], inputs["mu"], inputs["sigma"])
    nc = build(cfg, prep, scal)
    in_maps = make_in_maps(cfg, prep, inputs["h"], inputs["W_emb"], inputs["b_emb"],
                           inputs["Wg"], inputs["Wroot"], inputs["b_conv"],
                           inputs["W_out"], inputs["b_out"])
    res = bass_utils.run_bass_kernel_spmd(nc, in_maps, core_ids=list(range(cfg.C)),
                                          trace=trace)
    out = np.concatenate(
        [res.results[m]["out"][:cfg.B] for m in range(cfg.C)], axis=0)
    out = out.astype(np.float32) + np.asarray(inputs["b_out"], np.float32)[None, :]
    return out, res


def kernel(**inputs):
    cfg = Cfg()
    out, _ = run(cfg, inputs, trace=False)
    return out


# revision 72
# speedup vs baseline: 1.0029x; 1.0029x over previous
"""MoNet (GMMConv GNN) distributed Trainium2 kernel — source-partitioned.

Strategy (8 NeuronCores):
  - Edges partitioned by SOURCE core (row // 6250): each core computes xg for
    its local nodes only (no xg AllGather) and gathers source rows from its
    OWN small table (6272 rows, int16 indices, one SWDGE index space).
  - Per layer: local xg = h @ Wg written to a local DRAM table (bf16,
    128-col rows = 256B); edges sorted by global dest block (448 blocks of
    112 dests); per 128-edge tile one gaussian-weighted one-hot matmul
    scatters into a [96 feat x 112 dest] PSUM accumulator (transposed
    layout, so no PE transposes anywhere); per dest-group staging is DMAd to
    a partial-aggregate buffer [8*96, 6272] and a single bf16 ReduceScatter
    (add) replaces the baseline's 25MB AllGather.
  - Root weight + conv bias are folded into one K=97 matmul (h carries a
    ones row); the reduce-scattered aggregate is injected into the same PSUM
    via an identity matmul; epilogue is relu + residual add in-place.
  - All matmuls/tables bf16 (PSUM accumulation f32); gaussian edge
    coefficients computed in f32.
  - Host does index prep only: degree/dinv, edge bucketing/padding.
"""

import sys
from contextlib import ExitStack

import numpy as np

if "/opt/trn_rl_repo" not in sys.path:
    sys.path.insert(0, "/opt/trn_rl_repo")

import ml_dtypes

import concourse.bacc as bacc
import concourse.mybir as mybir
import concourse.tile as tile
from concourse import bass_utils, library_config

F32 = mybir.dt.float32
BF16 = mybir.dt.bfloat16
I16 = mybir.dt.int16
AF = mybir.ActivationFunctionType
ALU = mybir.AluOpType

P = 128
EPS = 1e-15
BF = ml_dtypes.bfloat16


class Cfg:
    def __init__(self):
        self.N, self.E = 50000, 800000
        self.NFEAT, self.NHID, self.NCLASS, self.NL, self.C = 128, 96, 40, 2, 8
        self.B = self.N // self.C            # 6250 real nodes per core
        self.BS = 224                        # dest block size
        self.BPG = 28                        # blocks per group (core)
        self.NPH = 7                         # RS pipeline phases
        self.HB = self.BPG // self.NPH       # 4 blocks per phase
        self.Bp = self.BS * self.BPG         # 6272 padded nodes per core
        self.Bh = self.BS * self.HB          # 1568 cols per phase
        self.NGB = self.C * self.BPG         # 448 global dest blocks
        self.NSB = self.Bp // P              # 49 source 128-blocks
        self.TPC = 7                         # gather tiles per SWDGE call
        self.RING = 49152                    # 3072-descriptor SWDGE ring
        self.SKIP_RS = False                 # debug: replace ReduceScatter
        self.SKIP_GATHER = False             # debug: skip dma_gather calls


def host_prep(cfg, edge_index, edge_weight):
    """Bucket edges by (dest half, source core, global dest block); pad tiles
    to the max count over cores so the SPMD program structure is uniform.
    Blocks are ordered (half, group, k) so each half's tiles are contiguous
    and the first half's ReduceScatter can overlap the second half's math."""
    N, C, B, BS, BPG, NGB = cfg.N, cfg.C, cfg.B, cfg.BS, cfg.BPG, cfg.NGB
    HB = cfg.HB
    row = np.asarray(edge_index[0]).astype(np.int64)
    col = np.asarray(edge_index[1]).astype(np.int64)
    ew = np.asarray(edge_weight).astype(np.float64)
    deg = np.bincount(row, weights=ew, minlength=N).astype(np.float64)
    with np.errstate(divide="ignore"):
        dinv = np.where(deg > 0, 1.0 / np.sqrt(deg), 0.0).astype(np.float32)

    core = row // B
    src_loc = row - core * B
    g = col // B
    dlg = col - g * B
    kblk = dlg // BS
    lane_d = (dlg - kblk * BS).astype(np.float32)
    ph = kblk // HB
    # phase-ordered slot: (phase, group, k within phase)
    gb = ph * (C * HB) + g * HB + (kblk - ph * HB)

    order = np.lexsort((gb, core))
    core, gb = core[order], gb[order]
    src_loc, lane_d = src_loc[order], lane_d[order]
    u = dinv[row[order]]
    v = dinv[col[order]]

    cnt = np.zeros((C, NGB), np.int64)
    np.add.at(cnt, (core, gb), 1)
    K = ((cnt + P - 1) // P).max(axis=0)          # tiles per slot
    toff = np.concatenate([[0], np.cumsum(K)]).astype(np.int64)
    T = int(toff[-1])

    gg = core * NGB + gb
    gcnt = np.bincount(gg, minlength=C * NGB)
    gstart = np.concatenate([[0], np.cumsum(gcnt)])[:-1]
    idx_in_g = np.arange(len(gg)) - gstart[gg]
    lane = (idx_in_g % P).astype(np.int64)
    t = (toff[gb] + idx_in_g // P).astype(np.int64)

    edA = np.zeros((C, P, 3 * T), np.float32)
    edA[:, :, 2 * T:3 * T] = -1.0                 # dl sentinel: no dest match
    edA[core, lane, t] = u
    edA[core, lane, T + t] = v
    edA[core, lane, 2 * T + t] = lane_d

    # int16 idx, wrapped-16: element (t, lane) at [lane % 16, t*8 + lane//16]
    idxA = np.zeros((C, 16, 8 * T), np.int16)     # pad idx 0 (valid row)
    idxA[core, lane % 16, t * 8 + lane // 16] = src_loc.astype(np.int16)
    idxA = np.tile(idxA, (1, 8, 1))
    return dict(idxA=idxA, edA=edA, K=[int(x) for x in K],
                toff=[int(x) for x in toff], T=T)


def make_scal(cfg, Wp, bp, mu, sigma):
    Wp = np.asarray(Wp, np.float64)
    bp = np.asarray(bp, np.float64)
    mu = np.asarray(mu, np.float64)
    sigma = np.asarray(sigma, np.float64)
    out = []
    for i in range(cfg.NL):
        out.append(dict(
            wp0=float(Wp[i, 0, 0]),
            wp1=float(Wp[i, 1, 0]),
            bp=float(bp[i, 0]),
            neg_mu=float(-mu[i, 0, 0]),
            s2inv=float(-0.5 / (EPS + sigma[i, 0, 0] ** 2)),
        ))
    return out


def build(cfg, prep, scal):
    NHID, NCLASS, NL, C = cfg.NHID, cfg.NCLASS, cfg.NL, cfg.C
    BS, BPG, Bp, NGB, NSB, TPC = cfg.BS, cfg.BPG, cfg.Bp, cfg.NGB, cfg.NSB, cfg.TPC
    HB, Bh = cfg.HB, cfg.Bh
    K2, toff, T = prep["K"], prep["toff"], prep["T"]

    nc = bacc.Bacc("TRN2", target_bir_lowering=False, debug=False,
                   num_devices=C, dynamic_dma_scratch_size=cfg.RING)
    hT_in = nc.declare_dram_parameter("hT", [P, Bp], BF16, isOutput=False)
    idx_in = nc.declare_dram_parameter("idx16", [P, 8 * T], I16, isOutput=False)
    ed_in = nc.declare_dram_parameter("ed", [P, 3 * T], F32, isOutput=False)
    ri_in = nc.declare_dram_parameter("riota", [P, BS], BF16, isOutput=False)
    Wemb_in = nc.declare_dram_parameter("Wemb", [P, NHID], BF16, isOutput=False)
    Wg_in = nc.declare_dram_parameter("WgP", [NL, NHID, P], BF16, isOutput=False)
    Wr_in = nc.declare_dram_parameter("Wr", [NL, NHID, NHID], BF16, isOutput=False)
    Wo_in = nc.declare_dram_parameter("Wo", [NHID, NCLASS], BF16, isOutput=False)
    bemb_in = nc.declare_dram_parameter("bembT", [NHID, 1], F32, isOutput=False)
    bconv_in = nc.declare_dram_parameter("bconvT", [NHID, NL], F32, isOutput=False)

    out_ext = nc.declare_dram_parameter("out", [Bp, NCLASS], F32, isOutput=True)

    with tile.TileContext(nc) as tc, ExitStack() as ctx:
        nc.gpsimd.load_library(library_config.mlp)
        const = ctx.enter_context(tc.tile_pool(name="const", bufs=1))
        hp = ctx.enter_context(tc.tile_pool(name="hp", bufs=2))
        gtp = ctx.enter_context(tc.tile_pool(name="gtp", bufs=2))
        gaussp = ctx.enter_context(tc.tile_pool(name="gaussp", bufs=1))
        xsp = ctx.enter_context(tc.tile_pool(name="xsp", bufs=1))
        xjp = ctx.enter_context(tc.tile_pool(name="xjp", bufs=4))
        selp = ctx.enter_context(tc.tile_pool(name="selp", bufs=16))
        stp = ctx.enter_context(tc.tile_pool(name="stp", bufs=3))
        agp = ctx.enter_context(tc.tile_pool(name="agp", bufs=4))
        rootp = ctx.enter_context(tc.tile_pool(name="rootp", bufs=2))
        rlp = ctx.enter_context(tc.tile_pool(name="rlp", bufs=4))
        obp = ctx.enter_context(tc.tile_pool(name="obp", bufs=1))
        pmm = ctx.enter_context(tc.tile_pool(name="pmm", bufs=3, space="PSUM"))
        pagg = ctx.enter_context(tc.tile_pool(name="pagg", bufs=4, space="PSUM"))
        prt = ctx.enter_context(tc.tile_pool(name="prt", bufs=1, space="PSUM"))
        dramp = ctx.enter_context(tc.tile_pool(name="dramp", bufs=1, space="DRAM"))

        def cload(ap, shape, dtype=F32, name=None):
            tl = const.tile(shape, dtype, name=name or "c")
            nc.sync.dma_start(out=tl[:], in_=ap)
            return tl

        hTin_s = cload(hT_in[:, :], [P, Bp], BF16, "hTin_s")
        idx_s = cload(idx_in[:, :], [P, 8 * T], I16, "idx_s")
        ed_s = cload(ed_in[:, :], [P, 3 * T], F32, "ed_s")
        u_s = ed_s[:, 0:T]
        v_s = ed_s[:, T:2 * T]
        dl_s = ed_s[:, 2 * T:3 * T]
        ri_s = cload(ri_in[:, :], [P, BS], BF16, "ri_s")
        Wemb_s = cload(Wemb_in[:, :], [P, NHID], BF16, "Wemb_s")
        bemb_s = cload(bemb_in[:, :], [NHID, 1], F32, "bemb_s")
        Wo_s = cload(Wo_in[:, :], [NHID, NCLASS], BF16, "Wo_s")
        bconv_s = cload(bconv_in[:, :], [NHID, NL], F32, "bconv_s")

        Wg_s = const.tile([NHID, NL * P], BF16, name="Wg_s")
        Wr_s = const.tile([NHID, NL * NHID], BF16, name="Wr_s")
        for i in range(NL):
            nc.sync.dma_start(out=Wg_s[:, i * P:(i + 1) * P], in_=Wg_in[i])
            nc.sync.dma_start(out=Wr_s[:, i * NHID:(i + 1) * NHID], in_=Wr_in[i])

        # ---- embedding: h0T[96, Bp] = (h @ Wemb + bemb).T ----
        # 4 node-blocks share one PSUM bank so each Act copy moves 512 cols.
        h_cur = hp.tile([NHID, Bp], BF16, tag="h", name="h0")
        for q in range(0, NSB, 4):
            nb = min(4, NSB - q)
            pe = pmm.tile([P, 4 * P], F32, tag="mm2", name="pe")
            for b in range(nb):
                c0 = (q + b) * P
                nc.tensor.matmul(pe[:NHID, b * P:(b + 1) * P], lhsT=Wemb_s[:],
                                 rhs=hTin_s[:, c0:c0 + P], start=True, stop=True)
            if (q // 4) % 2 == 0:
                nc.scalar.activation(out=h_cur[:, q * P:q * P + nb * P],
                                     in_=pe[:NHID, :nb * P],
                                     func=AF.Identity, bias=bemb_s[:, 0:1])
            else:
                nc.vector.tensor_scalar(out=h_cur[:, q * P:q * P + nb * P],
                                        in0=pe[:NHID, :nb * P],
                                        scalar1=bemb_s[:, 0:1], scalar2=None,
                                        op0=ALU.add)

        # ---- layers ----
        gauss_l = []
        pending_epi3 = None
        QSPL = 36  # xg/head blocks below this need only epi phases 0-2
        for li in range(NL):
            # local xg table -> DRAM [Bp, 128] bf16 (4 blocks per PSUM bank).
            # Emitted in two parts around the previous layer's phase-3
            # epilogue so part A runs while that layer's last RS is in flight.
            xgstage = xsp.tile([P, NSB * P], BF16, tag="xgs", name="xgs")

            def xg_part(qr, li=li, xgstage=xgstage, h_cur=h_cur):
                for q in qr:
                    nb = min(4, NSB - q)
                    px = pmm.tile([P, 4 * P], F32, tag="mm2", name="px")
                    for b in range(nb):
                        c0 = (q + b) * P
                        nc.tensor.matmul(px[:, b * P:(b + 1) * P],
                                         lhsT=h_cur[:, c0:c0 + P],
                                         rhs=Wg_s[:, li * P:(li + 1) * P],
                                         start=True, stop=True)
                    if (q // 4) % 2 == 0:
                        nc.scalar.copy(out=xgstage[:, q * P:q * P + nb * P],
                                       in_=px[:, :nb * P])
                    else:
                        nc.vector.tensor_copy(
                            out=xgstage[:, q * P:q * P + nb * P],
                            in_=px[:, :nb * P])

            xg_part(range(0, QSPL, 4))
            xg_d = dramp.tile([Bp, P], BF16, tag=f"xg{li}", name=f"xg{li}")
            nc.sync.dma_start(
                out=xg_d[0:QSPL * P, :].rearrange("(a p) c -> p a c", p=P),
                in_=xgstage[:, 0:QSPL * P].rearrange("p (a c) -> p a c", c=P))
            if pending_epi3 is not None:
                pending_epi3()
            xg_part(range(QSPL, NSB, 4))
            nc.sync.dma_start(
                out=xg_d[QSPL * P:, :].rearrange("(a p) c -> p a c", p=P),
                in_=xgstage[:, QSPL * P:].rearrange("p (a c) -> p a c", c=P))

            def emit_gauss(lj):
                # gaussian edge coefficients (only need ed); the elementwise
                # chain runs on the otherwise-idle gpsimd.
                if True:
                    sc = scal[lj]
                    t1 = gtp.tile([P, T], F32, tag="g1", name="g1")
                    nc.gpsimd.tensor_scalar(out=t1[:], in0=v_s[:],
                                            scalar1=sc["wp1"], scalar2=sc["bp"],
                                            op0=ALU.mult, op1=ALU.add)
                    t2 = gtp.tile([P, T], F32, tag="g2", name="g2")
                    nc.gpsimd.tensor_scalar(out=t2[:], in0=u_s[:],
                                            scalar1=sc["wp0"],
                                            scalar2=None, op0=ALU.mult)
                    t3 = gtp.tile([P, T], F32, tag="g1", name="g3")
                    nc.gpsimd.tensor_tensor(out=t3[:], in0=t1[:], in1=t2[:],
                                            op=ALU.add)
                    t4 = gtp.tile([P, T], F32, tag="g2", name="g4")
                    nc.scalar.activation(out=t4[:], in_=t3[:], func=AF.Tanh)
                    t4b = gtp.tile([P, T], F32, tag="g1", name="g4b")
                    nc.gpsimd.tensor_scalar(out=t4b[:], in0=t4[:],
                                            scalar1=sc["neg_mu"],
                                            scalar2=None, op0=ALU.add)
                    t5 = gtp.tile([P, T], F32, tag="g2", name="g5")
                    nc.scalar.activation(out=t5[:], in_=t4b[:], func=AF.Square)
                    g_s = gaussp.tile([P, T], F32, tag=f"gauss{lj}",
                                      name=f"gauss{lj}")
                    nc.scalar.activation(out=g_s[:], in_=t5[:], func=AF.Exp,
                                         scale=sc["s2inv"])
                    gauss_l.append(g_s)

            if li == 0:
                emit_gauss(0)
            gauss_s = gauss_l[li]

            # Per phase: gather calls emitted just before that phase's scatter
            # so the Pool stream reaches the RS instruction promptly and each
            # phase's RS overlaps the next phase's math. Epilogues are emitted
            # after ALL phases so they don't block the stream order.
            h_new = hp.tile([NHID, Bp], BF16, tag="h", name=f"h{li + 1}")
            tile_call = {}
            agg_half = []
            pending_rs = []

            def emit_rs():
                # deferred two phases so the Pool stream (which also carries
                # gather descriptor-gen) reaches the collective only after
                # its sem-wait on the flush DMAs is long satisfied — a
                # stalled Pool stops descriptor-gen and drains the DMA
                # pipeline. The SBUF load of the result is deferred to the
                # epilogue: a sync-queue DMA waiting on the collective would
                # hold SP.SEQ and block the later staging flushes.
                partial, hf_ = pending_rs.pop(0)
                if cfg.SKIP_RS:
                    agg_half.append(partial[0:NHID, :])
                    return
                aggrs_d = dramp.tile([NHID, Bh], BF16, tag=f"ag{li}h{hf_}",
                                     name=f"aggrs{li}h{hf_}")
                nc.gpsimd.collective_compute(
                    "ReduceScatter", ALU.add,
                    replica_groups=[list(range(C))],
                    ins=[partial[:, :]],
                    outs=[aggrs_d[:, :]],
                )
                agg_half.append(aggrs_d)

            for hf in range(cfg.NPH):
                ta = toff[hf * C * HB]
                tb = toff[(hf + 1) * C * HB]
                t0 = ta
                while t0 < tb:
                    kc = min(TPC, tb - t0)
                    xj = xjp.tile([P, TPC * P], BF16, tag="xj", name="xj")
                    if cfg.SKIP_GATHER:
                        nc.vector.memset(xj[:], 0.0)
                    else:
                        out_ap = xj[:, :kc * P].rearrange("p (k e) -> p k e", e=P)
                        nc.gpsimd.dma_gather(out_ap, xg_d[:, :],
                                             idx_s[:, t0 * 8:(t0 + kc) * 8],
                                             kc * P, kc * P, P)
                    for s in range(kc):
                        tile_call[t0 + s] = (xj, s)
                    t0 += kc
                while pending_rs:
                    emit_rs()

                partial_d = dramp.tile([C * NHID, Bh], BF16, tag=f"pt{li}h{hf}",
                                       name=f"partial{li}h{hf}")
                ncopy = 0
                for grp in range(C):
                    stg = stp.tile([NHID, Bh], BF16, tag="stg", name="stg")
                    for k in range(HB):
                        slot = hf * (C * HB) + grp * HB + k
                        Kb = K2[slot]
                        if Kb == 0:
                            nc.vector.memset(stg[:, k * BS:(k + 1) * BS], 0.0)
                            continue
                        pa = pagg.tile([NHID, BS], F32, tag="pa", name="pa")
                        for j in range(Kb):
                            t = toff[slot] + j
                            sel = selp.tile([P, BS], BF16, tag="sel", name="sel")
                            nc.vector.tensor_scalar(
                                out=sel[:], in0=ri_s[:],
                                scalar1=dl_s[:, t:t + 1],
                                scalar2=gauss_s[:, t:t + 1],
                                op0=ALU.is_equal, op1=ALU.mult)
                            xj, sl = tile_call[t]
                            nc.tensor.matmul(
                                pa[:, :],
                                lhsT=xj[:, sl * P:sl * P + NHID],
                                rhs=sel[:], start=(j == 0), stop=(j == Kb - 1))
                        # alternate copy engine 2:1 to balance Act/DVE load
                        if ncopy % 3 != 2:
                            nc.scalar.copy(out=stg[:, k * BS:(k + 1) * BS],
                                           in_=pa[:, :])
                        else:
                            nc.vector.tensor_copy(out=stg[:, k * BS:(k + 1) * BS],
                                                  in_=pa[:, :])
                        ncopy += 1
                    fl = nc.sync.dma_start(
                        out=partial_d[grp * NHID:(grp + 1) * NHID, :],
                        in_=stg[:, :])
                pending_rs.append((partial_d, hf))
                if li == 0 and hf == 0:
                    emit_gauss(1)   # layer-1 coefficients during the scatter
            last_flush = fl
            while pending_rs:
                emit_rs()

            # root term: rootT = (h @ Wroot + bconv).T — emitted after the
            # scatter so its Act copies don't clog the front of the Act
            # queue (they run while the reduce-scatters are in flight).
            root_sb = rootp.tile([NHID, Bp], BF16, tag="root", name=f"root{li}")
            for q in range(0, BPG, 2):
                pr = prt.tile([NHID, 2 * BS], F32, tag="rt", name="pr")
                for b in range(2):
                    c0 = (q + b) * BS
                    nc.tensor.matmul(pr[:, b * BS:(b + 1) * BS],
                                     lhsT=Wr_s[:, li * NHID:(li + 1) * NHID],
                                     rhs=h_cur[:, c0:c0 + BS],
                                     start=True, stop=True)
                nc.scalar.activation(out=root_sb[:, q * BS:(q + 2) * BS],
                                     in_=pr[:, :], func=AF.Identity,
                                     bias=bconv_s[:, li:li + 1])

            # epilogues (early phases overlap the later RSs); phase 3 is
            # deferred into the NEXT layer's xg section:
            # h_new = h_cur + relu(root + agg)
            def emit_epi(hf, agg_half=agg_half, root_sb=root_sb,
                         h_new=h_new, h_cur=h_cur, last_flush=last_flush):
                aggsb = agp.tile([NHID, Bh], BF16, tag="agg", name="aggsb")
                ld = nc.sync.dma_start(out=aggsb[:, :], in_=agg_half[hf][:, :])
                # order this load AFTER the last staging flush: it waits on
                # the collective while holding SP.SEQ, which would otherwise
                # block the remaining flush DMAs queued behind it.
                ld.ins.add_dependency(
                    last_flush.ins.name,
                    mybir.DependencyInfo(sync=True, no_sync=False))
                hc0 = hf * Bh
                for k in range(HB):
                    c0 = hc0 + k * BS
                    sm = rlp.tile([NHID, BS], BF16, tag="sm", name="sm")
                    nc.vector.tensor_tensor(out=sm[:, :],
                                            in0=aggsb[:, k * BS:(k + 1) * BS],
                                            in1=root_sb[:, c0:c0 + BS], op=ALU.add)
                    rl = rlp.tile([NHID, BS], BF16, tag="rl", name="rl")
                    if k % 2 == 0:
                        nc.scalar.activation(out=rl[:, :], in_=sm[:, :],
                                             func=AF.Relu)
                    else:
                        nc.vector.tensor_scalar(out=rl[:, :], in0=sm[:, :],
                                                scalar1=0.0, scalar2=None,
                                                op0=ALU.max)
                    nc.vector.tensor_tensor(out=h_new[:, c0:c0 + BS],
                                            in0=rl[:, :],
                                            in1=h_cur[:, c0:c0 + BS],
                                            op=ALU.add)

            for hf in range(cfg.NPH - 1):
                emit_epi(hf)
            pending_epi3 = lambda f=emit_epi: f(cfg.NPH - 1)
            h_cur = h_new

        # ---- output head (4 blocks per PSUM bank); split around the last
        # layer's deferred phase-3 epilogue ----
        ob = obp.tile([P, NSB * NCLASS], F32, tag="ob", name="ob")

        def head_part(qr):
            for q in qr:
                nb = min(4, NSB - q)
                po = pmm.tile([P, 4 * P], F32, tag="mm2", name="po")
                for b in range(nb):
                    c0 = (q + b) * P
                    nc.tensor.matmul(po[:, b * NCLASS:(b + 1) * NCLASS],
                                     lhsT=h_cur[:, c0:c0 + P], rhs=Wo_s[:],
                                     start=True, stop=True)
                if (q // 4) % 2 == 0:
                    nc.scalar.copy(out=ob[:, q * NCLASS:(q + nb) * NCLASS],
                                   in_=po[:, :nb * NCLASS])
                else:
                    nc.vector.tensor_copy(
                        out=ob[:, q * NCLASS:(q + nb) * NCLASS],
                        in_=po[:, :nb * NCLASS])

        head_part(range(0, QSPL, 4))
        pending_epi3()
        head_part(range(QSPL, NSB, 4))
        nc.sync.dma_start(
            out=out_ext[:, :].rearrange("(a p) c -> p a c", p=P),
            in_=ob[:, :].rearrange("p (a c) -> p a c", c=NCLASS))

    nc.finalize()
    return nc


def make_in_maps(cfg, prep, h, W_emb, b_emb, Wg, Wroot, b_conv, W_out, b_out):
    C, B, Bp, NL = cfg.C, cfg.B, cfg.Bp, cfg.NL
    NHID, NCLASS, BS, P_ = cfg.NHID, cfg.NCLASS, cfg.BS, P
    h = np.asarray(h, np.float32)
    WgP = np.zeros((NL, NHID, P_), np.float32)
    WgP[:, :, :NHID] = np.asarray(Wg, np.float32).reshape(NL, NHID, NHID)
    riota = np.tile(np.arange(BS, dtype=np.float32), (P_, 1))
    Wemb_f = np.asarray(W_emb, np.float32)
    bemb_f = np.asarray(b_emb, np.float32)
    Wcomb = Wemb_f @ WgP[0]
    xgb0 = np.concatenate([np.ones(P_, np.float32), bemb_f @ WgP[0]])[None, :]
    common = dict(
        riota=np.ascontiguousarray(riota.astype(BF)),
        Wemb=np.ascontiguousarray(np.asarray(W_emb, np.float32).astype(BF)),
        WgP=np.ascontiguousarray(WgP.astype(BF)),
        Wr=np.ascontiguousarray(np.asarray(Wroot, np.float32).astype(BF)),
        Wo=np.ascontiguousarray(np.asarray(W_out, np.float32).astype(BF)),
        bembT=np.ascontiguousarray(bemb_f[:, None]),
        bconvT=np.ascontiguousarray(np.asarray(b_conv, np.float32).T),
    )
    in_maps = []
    for m in range(C):
        d = dict(common)
        hT = np.zeros((P_, Bp), np.float32)
        hT[:, :B] = h[m * B:(m + 1) * B, :].T
        d["hT"] = np.ascontiguousarray(hT.astype(BF))
        d["idx16"] = np.ascontiguousarray(prep["idxA"][m])
        d["ed"] = np.ascontiguousarray(prep["edA"][m])
        in_maps.append(d)
    return in_maps


def run(cfg, inputs, trace=False):
    prep = host_prep(cfg, inputs["edge_index"], inputs["edge_weight"])
    scal = make_scal(cfg, inputs["Wp"], inputs["bp"], inputs["mu"], inputs["sigma"])
    nc = build(cfg, prep, scal)
    in_maps = make_in_maps(cfg, prep, inputs["h"], inputs["W_emb"], inputs["b_emb"],
                           inputs["Wg"], inputs["Wroot"], inputs["b_conv"],
                           inputs["W_out"], inputs["b_out"])
    res = bass_utils.run_bass_kernel_spmd(nc, in_maps, core_ids=list(range(cfg.C)),
                                          trace=trace)
    out = np.concatenate(
        [res.results[m]["out"][:cfg.B] for m in range(cfg.C)], axis=0)
    out = out.astype(np.float32) + np.asarray(inputs["b_out"], np.float32)[None, :]
    return out, res


def kernel(**inputs):
    cfg = Cfg()
    out, _ = run(cfg, inputs, trace=False)
    return out


# revision 74
# speedup vs baseline: 1.0135x; 1.0105x over previous
"""MoNet (GMMConv GNN) distributed Trainium2 kernel — source-partitioned.

Strategy (8 NeuronCores):
  - Edges partitioned by SOURCE core (row // 6250): each core computes xg for
    its local nodes only (no xg AllGather) and gathers source rows from its
    OWN small table (6272 rows, int16 indices, one SWDGE index space).
  - Per layer: local xg = h @ Wg written to a local DRAM table (bf16,
    128-col rows = 256B); edges sorted by global dest block (448 blocks of
    112 dests); per 128-edge tile one gaussian-weighted one-hot matmul
    scatters into a [96 feat x 112 dest] PSUM accumulator (transposed
    layout, so no PE transposes anywhere); per dest-group staging is DMAd to
    a partial-aggregate buffer [8*96, 6272] and a single bf16 ReduceScatter
    (add) replaces the baseline's 25MB AllGather.
  - Root weight + conv bias are folded into one K=97 matmul (h carries a
    ones row); the reduce-scattered aggregate is injected into the same PSUM
    via an identity matmul; epilogue is relu + residual add in-place.
  - All matmuls/tables bf16 (PSUM accumulation f32); gaussian edge
    coefficients computed in f32.
  - Host does index prep only: degree/dinv, edge bucketing/padding.
"""

import sys
from contextlib import ExitStack

import numpy as np

if "/opt/trn_rl_repo" not in sys.path:
    sys.path.insert(0, "/opt/trn_rl_repo")

import ml_dtypes

import concourse.bacc as bacc
import concourse.mybir as mybir
import concourse.tile as tile
from concourse import bass_utils, library_config

F32 = mybir.dt.float32
BF16 = mybir.dt.bfloat16
I16 = mybir.dt.int16
AF = mybir.ActivationFunctionType
ALU = mybir.AluOpType

P = 128
EPS = 1e-15
BF = ml_dtypes.bfloat16


class Cfg:
    def __init__(self):
        self.N, self.E = 50000, 800000
        self.NFEAT, self.NHID, self.NCLASS, self.NL, self.C = 128, 96, 40, 2, 8
        self.B = self.N // self.C            # 6250 real nodes per core
        self.BS = 224                        # dest block size
        self.BPG = 28                        # blocks per group (core)
        self.NPH = 7                         # RS pipeline phases
        self.HB = self.BPG // self.NPH       # 4 blocks per phase
        self.Bp = self.BS * self.BPG         # 6272 padded nodes per core
        self.Bh = self.BS * self.HB          # 1568 cols per phase
        self.NGB = self.C * self.BPG         # 448 global dest blocks
        self.NSB = self.Bp // P              # 49 source 128-blocks
        self.TPC = 7                         # gather tiles per SWDGE call
        self.RING = 49152                    # 3072-descriptor SWDGE ring
        self.SKIP_RS = False                 # debug: replace ReduceScatter
        self.SKIP_GATHER = False             # debug: skip dma_gather calls


def host_prep(cfg, edge_index, edge_weight):
    """Bucket edges by (dest half, source core, global dest block); pad tiles
    to the max count over cores so the SPMD program structure is uniform.
    Blocks are ordered (half, group, k) so each half's tiles are contiguous
    and the first half's ReduceScatter can overlap the second half's math."""
    N, C, B, BS, BPG, NGB = cfg.N, cfg.C, cfg.B, cfg.BS, cfg.BPG, cfg.NGB
    HB = cfg.HB
    row = np.asarray(edge_index[0]).astype(np.int64)
    col = np.asarray(edge_index[1]).astype(np.int64)
    ew = np.asarray(edge_weight).astype(np.float64)
    deg = np.bincount(row, weights=ew, minlength=N).astype(np.float64)
    with np.errstate(divide="ignore"):
        dinv = np.where(deg > 0, 1.0 / np.sqrt(deg), 0.0).astype(np.float32)

    core = row // B
    src_loc = row - core * B
    g = col // B
    dlg = col - g * B
    kblk = dlg // BS
    lane_d = (dlg - kblk * BS).astype(np.float32)
    ph = kblk // HB
    # phase-ordered slot: (phase, group, k within phase)
    gb = ph * (C * HB) + g * HB + (kblk - ph * HB)

    order = np.lexsort((gb, core))
    core, gb = core[order], gb[order]
    src_loc, lane_d = src_loc[order], lane_d[order]
    u = dinv[row[order]]
    v = dinv[col[order]]

    cnt = np.zeros((C, NGB), np.int64)
    np.add.at(cnt, (core, gb), 1)
    K = ((cnt + P - 1) // P).max(axis=0)          # tiles per slot
    toff = np.concatenate([[0], np.cumsum(K)]).astype(np.int64)
    T = int(toff[-1])

    gg = core * NGB + gb
    gcnt = np.bincount(gg, minlength=C * NGB)
    gstart = np.concatenate([[0], np.cumsum(gcnt)])[:-1]
    idx_in_g = np.arange(len(gg)) - gstart[gg]
    lane = (idx_in_g % P).astype(np.int64)
    t = (toff[gb] + idx_in_g // P).astype(np.int64)

    edA = np.zeros((C, P, 3 * T), np.float32)
    edA[:, :, 2 * T:3 * T] = -1.0                 # dl sentinel: no dest match
    edA[core, lane, t] = u
    edA[core, lane, T + t] = v
    edA[core, lane, 2 * T + t] = lane_d

    # int16 idx, wrapped-16: element (t, lane) at [lane % 16, t*8 + lane//16]
    idxA = np.zeros((C, 16, 8 * T), np.int16)     # pad idx 0 (valid row)
    idxA[core, lane % 16, t * 8 + lane // 16] = src_loc.astype(np.int16)
    idxA = np.tile(idxA, (1, 8, 1))
    return dict(idxA=idxA, edA=edA, K=[int(x) for x in K],
                toff=[int(x) for x in toff], T=T)


def make_scal(cfg, Wp, bp, mu, sigma):
    Wp = np.asarray(Wp, np.float64)
    bp = np.asarray(bp, np.float64)
    mu = np.asarray(mu, np.float64)
    sigma = np.asarray(sigma, np.float64)
    out = []
    for i in range(cfg.NL):
        out.append(dict(
            wp0=float(Wp[i, 0, 0]),
            wp1=float(Wp[i, 1, 0]),
            bp=float(bp[i, 0]),
            neg_mu=float(-mu[i, 0, 0]),
            s2inv=float(-0.5 / (EPS + sigma[i, 0, 0] ** 2)),
        ))
    return out


def build(cfg, prep, scal):
    NHID, NCLASS, NL, C = cfg.NHID, cfg.NCLASS, cfg.NL, cfg.C
    BS, BPG, Bp, NGB, NSB, TPC = cfg.BS, cfg.BPG, cfg.Bp, cfg.NGB, cfg.NSB, cfg.TPC
    HB, Bh = cfg.HB, cfg.Bh
    K2, toff, T = prep["K"], prep["toff"], prep["T"]

    nc = bacc.Bacc("TRN2", target_bir_lowering=False, debug=False,
                   num_devices=C, dynamic_dma_scratch_size=cfg.RING)
    hT_in = nc.declare_dram_parameter("hT", [P, Bp], BF16, isOutput=False)
    idx_in = nc.declare_dram_parameter("idx16", [P, 8 * T], I16, isOutput=False)
    ed_in = nc.declare_dram_parameter("ed", [P, 3 * T], F32, isOutput=False)
    ri_in = nc.declare_dram_parameter("riota", [P, BS], BF16, isOutput=False)
    Wemb_in = nc.declare_dram_parameter("Wemb", [P, NHID], BF16, isOutput=False)
    Wg_in = nc.declare_dram_parameter("WgP", [NL, NHID, P], BF16, isOutput=False)
    Wr_in = nc.declare_dram_parameter("Wr", [NL, NHID, NHID], BF16, isOutput=False)
    Wo_in = nc.declare_dram_parameter("Wo", [NHID, NCLASS], BF16, isOutput=False)
    bemb_in = nc.declare_dram_parameter("bembT", [NHID, 1], F32, isOutput=False)
    bconv_in = nc.declare_dram_parameter("bconvT", [NHID, NL], F32, isOutput=False)

    out_ext = nc.declare_dram_parameter("out", [Bp, NCLASS], F32, isOutput=True)

    with tile.TileContext(nc) as tc, ExitStack() as ctx:
        nc.gpsimd.load_library(library_config.mlp)
        const = ctx.enter_context(tc.tile_pool(name="const", bufs=1))
        hp = ctx.enter_context(tc.tile_pool(name="hp", bufs=2))
        gtp = ctx.enter_context(tc.tile_pool(name="gtp", bufs=2))
        gaussp = ctx.enter_context(tc.tile_pool(name="gaussp", bufs=1))
        xsp = ctx.enter_context(tc.tile_pool(name="xsp", bufs=1))
        xjp = ctx.enter_context(tc.tile_pool(name="xjp", bufs=6))
        selp = ctx.enter_context(tc.tile_pool(name="selp", bufs=16))
        stp = ctx.enter_context(tc.tile_pool(name="stp", bufs=4))
        agp = ctx.enter_context(tc.tile_pool(name="agp", bufs=4))
        rootp = ctx.enter_context(tc.tile_pool(name="rootp", bufs=2))
        rlp = ctx.enter_context(tc.tile_pool(name="rlp", bufs=4))
        obp = ctx.enter_context(tc.tile_pool(name="obp", bufs=1))
        pmm = ctx.enter_context(tc.tile_pool(name="pmm", bufs=2, space="PSUM"))
        pagg = ctx.enter_context(tc.tile_pool(name="pagg", bufs=5, space="PSUM"))
        prt = ctx.enter_context(tc.tile_pool(name="prt", bufs=1, space="PSUM"))
        dramp = ctx.enter_context(tc.tile_pool(name="dramp", bufs=1, space="DRAM"))

        def cload(ap, shape, dtype=F32, name=None):
            tl = const.tile(shape, dtype, name=name or "c")
            nc.sync.dma_start(out=tl[:], in_=ap)
            return tl

        hTin_s = cload(hT_in[:, :], [P, Bp], BF16, "hTin_s")
        idx_s = cload(idx_in[:, :], [P, 8 * T], I16, "idx_s")
        ed_s = cload(ed_in[:, :], [P, 3 * T], F32, "ed_s")
        u_s = ed_s[:, 0:T]
        v_s = ed_s[:, T:2 * T]
        dl_s = ed_s[:, 2 * T:3 * T]
        ri_s = cload(ri_in[:, :], [P, BS], BF16, "ri_s")
        Wemb_s = cload(Wemb_in[:, :], [P, NHID], BF16, "Wemb_s")
        bemb_s = cload(bemb_in[:, :], [NHID, 1], F32, "bemb_s")
        Wo_s = cload(Wo_in[:, :], [NHID, NCLASS], BF16, "Wo_s")
        bconv_s = cload(bconv_in[:, :], [NHID, NL], F32, "bconv_s")

        Wg_s = const.tile([NHID, NL * P], BF16, name="Wg_s")
        Wr_s = const.tile([NHID, NL * NHID], BF16, name="Wr_s")
        for i in range(NL):
            nc.sync.dma_start(out=Wg_s[:, i * P:(i + 1) * P], in_=Wg_in[i])
            nc.sync.dma_start(out=Wr_s[:, i * NHID:(i + 1) * NHID], in_=Wr_in[i])

        # ---- embedding: h0T[96, Bp] = (h @ Wemb + bemb).T ----
        # 4 node-blocks share one PSUM bank so each Act copy moves 512 cols.
        h_cur = hp.tile([NHID, Bp], BF16, tag="h", name="h0")
        for q in range(0, NSB, 4):
            nb = min(4, NSB - q)
            pe = pmm.tile([P, 4 * P], F32, tag="mm2", name="pe")
            for b in range(nb):
                c0 = (q + b) * P
                nc.tensor.matmul(pe[:NHID, b * P:(b + 1) * P], lhsT=Wemb_s[:],
                                 rhs=hTin_s[:, c0:c0 + P], start=True, stop=True)
            if (q // 4) % 2 == 0:
                nc.scalar.activation(out=h_cur[:, q * P:q * P + nb * P],
                                     in_=pe[:NHID, :nb * P],
                                     func=AF.Identity, bias=bemb_s[:, 0:1])
            else:
                nc.vector.tensor_scalar(out=h_cur[:, q * P:q * P + nb * P],
                                        in0=pe[:NHID, :nb * P],
                                        scalar1=bemb_s[:, 0:1], scalar2=None,
                                        op0=ALU.add)

        # ---- layers ----
        gauss_l = []
        pending_epi3 = None
        QSPL = 36  # xg/head blocks below this need only epi phases 0-2
        for li in range(NL):
            # local xg table -> DRAM [Bp, 128] bf16 (4 blocks per PSUM bank).
            # Emitted in two parts around the previous layer's phase-3
            # epilogue so part A runs while that layer's last RS is in flight.
            xgstage = xsp.tile([P, NSB * P], BF16, tag="xgs", name="xgs")

            def xg_part(qr, li=li, xgstage=xgstage, h_cur=h_cur):
                for q in qr:
                    nb = min(4, NSB - q)
                    px = pmm.tile([P, 4 * P], F32, tag="mm2", name="px")
                    for b in range(nb):
                        c0 = (q + b) * P
                        nc.tensor.matmul(px[:, b * P:(b + 1) * P],
                                         lhsT=h_cur[:, c0:c0 + P],
                                         rhs=Wg_s[:, li * P:(li + 1) * P],
                                         start=True, stop=True)
                    if (q // 4) % 2 == 0:
                        nc.scalar.copy(out=xgstage[:, q * P:q * P + nb * P],
                                       in_=px[:, :nb * P])
                    else:
                        nc.vector.tensor_copy(
                            out=xgstage[:, q * P:q * P + nb * P],
                            in_=px[:, :nb * P])

            xg_part(range(0, QSPL, 4))
            xg_d = dramp.tile([Bp, P], BF16, tag=f"xg{li}", name=f"xg{li}")
            nc.sync.dma_start(
                out=xg_d[0:QSPL * P, :].rearrange("(a p) c -> p a c", p=P),
                in_=xgstage[:, 0:QSPL * P].rearrange("p (a c) -> p a c", c=P))
            if pending_epi3 is not None:
                pending_epi3()
            xg_part(range(QSPL, NSB, 4))
            nc.sync.dma_start(
                out=xg_d[QSPL * P:, :].rearrange("(a p) c -> p a c", p=P),
                in_=xgstage[:, QSPL * P:].rearrange("p (a c) -> p a c", c=P))

            def emit_gauss(lj):
                # gaussian edge coefficients (only need ed); the elementwise
                # chain runs on the otherwise-idle gpsimd.
                if True:
                    sc = scal[lj]
                    t1 = gtp.tile([P, T], F32, tag="g1", name="g1")
                    nc.gpsimd.tensor_scalar(out=t1[:], in0=v_s[:],
                                            scalar1=sc["wp1"], scalar2=sc["bp"],
                                            op0=ALU.mult, op1=ALU.add)
                    t2 = gtp.tile([P, T], F32, tag="g2", name="g2")
                    nc.gpsimd.tensor_scalar(out=t2[:], in0=u_s[:],
                                            scalar1=sc["wp0"],
                                            scalar2=None, op0=ALU.mult)
                    t3 = gtp.tile([P, T], F32, tag="g1", name="g3")
                    nc.gpsimd.tensor_tensor(out=t3[:], in0=t1[:], in1=t2[:],
                                            op=ALU.add)
                    t4 = gtp.tile([P, T], F32, tag="g2", name="g4")
                    nc.scalar.activation(out=t4[:], in_=t3[:], func=AF.Tanh)
                    t4b = gtp.tile([P, T], F32, tag="g1", name="g4b")
                    nc.gpsimd.tensor_scalar(out=t4b[:], in0=t4[:],
                                            scalar1=sc["neg_mu"],
                                            scalar2=None, op0=ALU.add)
                    t5 = gtp.tile([P, T], F32, tag="g2", name="g5")
                    nc.scalar.activation(out=t5[:], in_=t4b[:], func=AF.Square)
                    g_s = gaussp.tile([P, T], F32, tag=f"gauss{lj}",
                                      name=f"gauss{lj}")
                    nc.scalar.activation(out=g_s[:], in_=t5[:], func=AF.Exp,
                                         scale=sc["s2inv"])
                    gauss_l.append(g_s)

            if li == 0:
                emit_gauss(0)
            gauss_s = gauss_l[li]

            # Per phase: gather calls emitted just before that phase's scatter
            # so the Pool stream reaches the RS instruction promptly and each
            # phase's RS overlaps the next phase's math. Epilogues are emitted
            # after ALL phases so they don't block the stream order.
            h_new = hp.tile([NHID, Bp], BF16, tag="h", name=f"h{li + 1}")
            tile_call = {}
            agg_half = []
            pending_rs = []

            def emit_rs():
                # deferred two phases so the Pool stream (which also carries
                # gather descriptor-gen) reaches the collective only after
                # its sem-wait on the flush DMAs is long satisfied — a
                # stalled Pool stops descriptor-gen and drains the DMA
                # pipeline. The SBUF load of the result is deferred to the
                # epilogue: a sync-queue DMA waiting on the collective would
                # hold SP.SEQ and block the later staging flushes.
                partial, hf_ = pending_rs.pop(0)
                if cfg.SKIP_RS:
                    agg_half.append(partial[0:NHID, :])
                    return
                aggrs_d = dramp.tile([NHID, Bh], BF16, tag=f"ag{li}h{hf_}",
                                     name=f"aggrs{li}h{hf_}")
                nc.gpsimd.collective_compute(
                    "ReduceScatter", ALU.add,
                    replica_groups=[list(range(C))],
                    ins=[partial[:, :]],
                    outs=[aggrs_d[:, :]],
                )
                agg_half.append(aggrs_d)

            for hf in range(cfg.NPH):
                ta = toff[hf * C * HB]
                tb = toff[(hf + 1) * C * HB]
                t0 = ta
                while t0 < tb:
                    kc = min(TPC, tb - t0)
                    xj = xjp.tile([P, TPC * P], BF16, tag="xj", name="xj")
                    if cfg.SKIP_GATHER:
                        nc.vector.memset(xj[:], 0.0)
                    else:
                        out_ap = xj[:, :kc * P].rearrange("p (k e) -> p k e", e=P)
                        nc.gpsimd.dma_gather(out_ap, xg_d[:, :],
                                             idx_s[:, t0 * 8:(t0 + kc) * 8],
                                             kc * P, kc * P, P)
                    for s in range(kc):
                        tile_call[t0 + s] = (xj, s)
                    t0 += kc
                while pending_rs:
                    emit_rs()

                partial_d = dramp.tile([C * NHID, Bh], BF16, tag=f"pt{li}h{hf}",
                                       name=f"partial{li}h{hf}")
                ncopy = 0
                for grp in range(C):
                    stg = stp.tile([NHID, Bh], BF16, tag="stg", name="stg")
                    for k in range(HB):
                        slot = hf * (C * HB) + grp * HB + k
                        Kb = K2[slot]
                        if Kb == 0:
                            nc.vector.memset(stg[:, k * BS:(k + 1) * BS], 0.0)
                            continue
                        pa = pagg.tile([NHID, BS], F32, tag="pa", name="pa")
                        for j in range(Kb):
                            t = toff[slot] + j
                            sel = selp.tile([P, BS], BF16, tag="sel", name="sel")
                            nc.vector.tensor_scalar(
                                out=sel[:], in0=ri_s[:],
                                scalar1=dl_s[:, t:t + 1],
                                scalar2=gauss_s[:, t:t + 1],
                                op0=ALU.is_equal, op1=ALU.mult)
                            xj, sl = tile_call[t]
                            nc.tensor.matmul(
                                pa[:, :],
                                lhsT=xj[:, sl * P:sl * P + NHID],
                                rhs=sel[:], start=(j == 0), stop=(j == Kb - 1))
                        # alternate copy engine 2:1 to balance Act/DVE load
                        if ncopy % 3 != 2:
                            nc.scalar.copy(out=stg[:, k * BS:(k + 1) * BS],
                                           in_=pa[:, :])
                        else:
                            nc.vector.tensor_copy(out=stg[:, k * BS:(k + 1) * BS],
                                                  in_=pa[:, :])
                        ncopy += 1
                    fl = nc.sync.dma_start(
                        out=partial_d[grp * NHID:(grp + 1) * NHID, :],
                        in_=stg[:, :])
                pending_rs.append((partial_d, hf))
                if li == 0 and hf == 0:
                    emit_gauss(1)   # layer-1 coefficients during the scatter
            last_flush = fl
            while pending_rs:
                emit_rs()

            # root term: rootT = (h @ Wroot + bconv).T — emitted after the
            # scatter so its Act copies don't clog the front of the Act
            # queue (they run while the reduce-scatters are in flight).
            root_sb = rootp.tile([NHID, Bp], BF16, tag="root", name=f"root{li}")
            for q in range(0, BPG, 2):
                pr = prt.tile([NHID, 2 * BS], F32, tag="rt", name="pr")
                for b in range(2):
                    c0 = (q + b) * BS
                    nc.tensor.matmul(pr[:, b * BS:(b + 1) * BS],
                                     lhsT=Wr_s[:, li * NHID:(li + 1) * NHID],
                                     rhs=h_cur[:, c0:c0 + BS],
                                     start=True, stop=True)
                nc.scalar.activation(out=root_sb[:, q * BS:(q + 2) * BS],
                                     in_=pr[:, :], func=AF.Identity,
                                     bias=bconv_s[:, li:li + 1])

            # epilogues (early phases overlap the later RSs); phase 3 is
            # deferred into the NEXT layer's xg section:
            # h_new = h_cur + relu(root + agg)
            def emit_epi(hf, agg_half=agg_half, root_sb=root_sb,
                         h_new=h_new, h_cur=h_cur, last_flush=last_flush):
                aggsb = agp.tile([NHID, Bh], BF16, tag="agg", name="aggsb")
                ld = nc.sync.dma_start(out=aggsb[:, :], in_=agg_half[hf][:, :])
                # order this load AFTER the last staging flush: it waits on
                # the collective while holding SP.SEQ, which would otherwise
                # block the remaining flush DMAs queued behind it.
                ld.ins.add_dependency(
                    last_flush.ins.name,
                    mybir.DependencyInfo(sync=True, no_sync=False))
                hc0 = hf * Bh
                for k in range(HB):
                    c0 = hc0 + k * BS
                    sm = rlp.tile([NHID, BS], BF16, tag="sm", name="sm")
                    nc.vector.tensor_tensor(out=sm[:, :],
                                            in0=aggsb[:, k * BS:(k + 1) * BS],
                                            in1=root_sb[:, c0:c0 + BS], op=ALU.add)
                    rl = rlp.tile([NHID, BS], BF16, tag="rl", name="rl")
                    if k % 2 == 0:
                        nc.scalar.activation(out=rl[:, :], in_=sm[:, :],
                                             func=AF.Relu)
                    else:
                        nc.vector.tensor_scalar(out=rl[:, :], in0=sm[:, :],
                                                scalar1=0.0, scalar2=None,
                                                op0=ALU.max)
                    nc.vector.tensor_tensor(out=h_new[:, c0:c0 + BS],
                                            in0=rl[:, :],
                                            in1=h_cur[:, c0:c0 + BS],
                                            op=ALU.add)

            for hf in range(cfg.NPH - 1):
                emit_epi(hf)
            pending_epi3 = lambda f=emit_epi: f(cfg.NPH - 1)
            h_cur = h_new

        # ---- output head (4 blocks per PSUM bank); split around the last
        # layer's deferred phase-3 epilogue ----
        ob = obp.tile([P, NSB * NCLASS], F32, tag="ob", name="ob")

        def head_part(qr):
            for q in qr:
                nb = min(4, NSB - q)
                po = pmm.tile([P, 4 * P], F32, tag="mm2", name="po")
                for b in range(nb):
                    c0 = (q + b) * P
                    nc.tensor.matmul(po[:, b * NCLASS:(b + 1) * NCLASS],
                                     lhsT=h_cur[:, c0:c0 + P], rhs=Wo_s[:],
                                     start=True, stop=True)
                if (q // 4) % 2 == 0:
                    nc.scalar.copy(out=ob[:, q * NCLASS:(q + nb) * NCLASS],
                                   in_=po[:, :nb * NCLASS])
                else:
                    nc.vector.tensor_copy(
                        out=ob[:, q * NCLASS:(q + nb) * NCLASS],
                        in_=po[:, :nb * NCLASS])

        head_part(range(0, QSPL, 4))
        pending_epi3()
        head_part(range(QSPL, NSB, 4))
        nc.sync.dma_start(
            out=out_ext[:, :].rearrange("(a p) c -> p a c", p=P),
            in_=ob[:, :].rearrange("p (a c) -> p a c", c=NCLASS))

    nc.finalize()
    return nc


def make_in_maps(cfg, prep, h, W_emb, b_emb, Wg, Wroot, b_conv, W_out, b_out):
    C, B, Bp, NL = cfg.C, cfg.B, cfg.Bp, cfg.NL
    NHID, NCLASS, BS, P_ = cfg.NHID, cfg.NCLASS, cfg.BS, P
    h = np.asarray(h, np.float32)
    WgP = np.zeros((NL, NHID, P_), np.float32)
    WgP[:, :, :NHID] = np.asarray(Wg, np.float32).reshape(NL, NHID, NHID)
    riota = np.tile(np.arange(BS, dtype=np.float32), (P_, 1))
    Wemb_f = np.asarray(W_emb, np.float32)
    bemb_f = np.asarray(b_emb, np.float32)
    Wcomb = Wemb_f @ WgP[0]
    xgb0 = np.concatenate([np.ones(P_, np.float32), bemb_f @ WgP[0]])[None, :]
    common = dict(
        riota=np.ascontiguousarray(riota.astype(BF)),
        Wemb=np.ascontiguousarray(np.asarray(W_emb, np.float32).astype(BF)),
        WgP=np.ascontiguousarray(WgP.astype(BF)),
        Wr=np.ascontiguousarray(np.asarray(Wroot, np.float32).astype(BF)),
        Wo=np.ascontiguousarray(np.asarray(W_out, np.float32).astype(BF)),
        bembT=np.ascontiguousarray(bemb_f[:, None]),
        bconvT=np.ascontiguousarray(np.asarray(b_conv, np.float32).T),
    )
    in_maps = []
    for m in range(C):
        d = dict(common)
        hT = np.zeros((P_, Bp), np.float32)
        hT[:, :B] = h[m * B:(m + 1) * B, :].T
        d["hT"] = np.ascontiguousarray(hT.astype(BF))
        d["idx16"] = np.ascontiguousarray(prep["idxA"][m])
        d["ed"] = np.ascontiguousarray(prep["edA"][m])
        in_maps.append(d)
    return in_maps


def run(cfg, inputs, trace=False):
    prep = host_prep(cfg, inputs["edge_index"], inputs["edge_weight"])
    scal = make_scal(cfg, inputs["Wp"], inputs["bp"], inputs["mu"], inputs["sigma"])
    nc = build(cfg, prep, scal)
    in_maps = make_in_maps(cfg, prep, inputs["h"], inputs["W_emb"], inputs["b_emb"],
                           inputs["Wg"], inputs["Wroot"], inputs["b_conv"],
                           inputs["W_out"], inputs["b_out"])
    res = bass_utils.run_bass_kernel_spmd(nc, in_maps, core_ids=list(range(cfg.C)),
                                          trace=trace)
    out = np.concatenate(
        [res.results[m]["out"][:cfg.B] for m in range(cfg.C)], axis=0)
    out = out.astype(np.float32) + np.asarray(inputs["b_out"], np.float32)[None, :]
    return out, res


def kernel(**inputs):
    cfg = Cfg()
    out, _ = run(cfg, inputs, trace=False)
    return out
